# revision 3
# baseline (speedup 1.0000x reference)
"""DeepKoopmanSplit TRN2 Bass kernel — data-parallel over 8 NeuronCores.

Shapes (hardcoded): B=8192, M=64, H=256, LP=32, LR=96, L=128, CD=4.
Per core: B_c=1024 batch rows, R=65536 (b,m) rows.

Strategy: feature-major (transposed) activations so MLP weights are the
PE-stationary operand; float32r matmuls (1 cyc/row); ACT/DVE split for
PSUM->SBUF bias+ReLU epilogues; PE transposes (via identity) for all
layout conversions back to row-major outputs.

Encoder output z is kept split (z_pos [32,*], z_rest [96,*], both at
partition base 0) because PSUM matmul outputs and engine operands must
start at partition 0; the Koopman scan produces a unified z [128,*].
"""
import numpy as np

import concourse.bass as bass
import concourse.mybir as mybir
import concourse.tile as tile
from concourse import bacc
from concourse.bass_utils import run_bass_kernel_spmd
from concourse.masks import make_identity

F32 = mybir.dt.float32
F32R = mybir.dt.float32r
AF = mybir.ActivationFunctionType
ALU = mybir.AluOpType

B, M = 8192, 64
H = 256
LP, LR = 32, 96
L = LP + LR  # 128
CD = 4
NCORES = 8
BC = B // NCORES          # 1024 batch rows / core
RSEQ = BC * M             # 65536 seq rows / core
TS = 512                  # rows per tile
NT_SEQ = RSEQ // TS       # 128 seq tiles
NT_XK = BC // TS          # 2 x_k tiles
NBB = BC // TS            # 2 scan b-blocks


def _round_f32r(a):
    u = np.ascontiguousarray(a, dtype=np.float32).view(np.uint32).astype(np.uint64)
    u = (u + 0x800) & 0xFFFFF000
    return u.astype(np.uint32).view(np.float32)


class _EngineBalancer:
    """Alternate PSUM->SBUF epilogues between ACT and DVE by running cost."""

    def __init__(self, nc):
        self.nc = nc
        self.act_ns = 0.0
        self.dve_ns = 0.0

    def ep(self, out, in_, relu=False, bias=None):
        fd = 1
        for s in in_.shape[1:]:
            fd *= s
        act_cost = (172 + fd) / 1.2
        dve_cost = (120 + fd) / 0.96
        use_act = (self.act_ns + act_cost) <= (self.dve_ns + dve_cost)
        if use_act:
            self.act_ns += act_cost
            if relu:
                self.nc.scalar.activation(out=out, in_=in_, func=AF.Relu,
                                          bias=0.0 if bias is None else bias)
            elif bias is not None:
                self.nc.scalar.activation(out=out, in_=in_, func=AF.Identity,
                                          bias=bias)
            else:
                self.nc.scalar.copy(out=out, in_=in_)
        else:
            self.dve_ns += dve_cost
            if relu:
                if bias is not None:
                    self.nc.vector.tensor_scalar(out=out, in0=in_, scalar1=bias,
                                                 scalar2=0.0, op0=ALU.add,
                                                 op1=ALU.max)
                else:
                    self.nc.vector.tensor_scalar(out=out, in0=in_, scalar1=0.0,
                                                 scalar2=None, op0=ALU.max)
            elif bias is not None:
                self.nc.vector.tensor_scalar(out=out, in0=in_, scalar1=bias,
                                             scalar2=None, op0=ALU.add)
            else:
                self.nc.vector.tensor_copy(out=out, in_=in_)


def build_module(nt_seq=NT_SEQ, n_scan=M):
    nc = bacc.Bacc()

    # ---- DRAM I/O ----
    xT_d = nc.dram_tensor("xT", [13, RSEQ], F32R, kind="ExternalInput")
    xkT_d = nc.dram_tensor("xkT", [13, BC], F32R, kind="ExternalInput")
    uT_d = nc.dram_tensor("uT", [CD, M, BC], F32R, kind="ExternalInput")
    w0e_d = nc.dram_tensor("w0e", [13, 512], F32R, kind="ExternalInput")
    w1e_d = nc.dram_tensor("w1e", [128, 4, 2, 128], F32R, kind="ExternalInput")
    w2ep_d = nc.dram_tensor("w2ep", [128, 2, LP], F32R, kind="ExternalInput")
    w2er_d = nc.dram_tensor("w2er", [128, 2, LR], F32R, kind="ExternalInput")
    # dec w0 for split z (from encoder) and unified z (from scan)
    w0dp_d = nc.dram_tensor("w0dp", [LP, 2, 128], F32R, kind="ExternalInput")
    w0dr_d = nc.dram_tensor("w0dr", [LR, 2, 128], F32R, kind="ExternalInput")
    w0du_d = nc.dram_tensor("w0du", [128, 512], F32R, kind="ExternalInput")
    w1d_d = nc.dram_tensor("w1d", [128, 4, 2, 128], F32R, kind="ExternalInput")
    w2dp_d = nc.dram_tensor("w2dp", [128, 2, 3], F32R, kind="ExternalInput")
    w2dr_d = nc.dram_tensor("w2dr", [128, 2, 9], F32R, kind="ExternalInput")
    aT_d = nc.dram_tensor("aT", [L, L], F32R, kind="ExternalInput")
    bT_d = nc.dram_tensor("bT", [CD, L], F32R, kind="ExternalInput")
    b_h1e_d = nc.dram_tensor("b_h1e", [128, 4], F32, kind="ExternalInput")
    b_zp_d = nc.dram_tensor("b_zp", [LP, 1], F32, kind="ExternalInput")
    b_zr_d = nc.dram_tensor("b_zr", [LR, 1], F32, kind="ExternalInput")
    b_h0d_d = nc.dram_tensor("b_h0d", [128, 4], F32, kind="ExternalInput")
    b_h1d_d = nc.dram_tensor("b_h1d", [128, 4], F32, kind="ExternalInput")
    b_xhp_d = nc.dram_tensor("b_xhp", [3, 1], F32, kind="ExternalInput")
    b_xhr_d = nc.dram_tensor("b_xhr", [9, 1], F32, kind="ExternalInput")

    xkh_d = nc.dram_tensor("xkh", [BC, 12], F32, kind="ExternalOutput")
    xth_d = nc.dram_tensor("xth", [RSEQ, 12], F32, kind="ExternalOutput")
    zp_d = nc.dram_tensor("zp", [BC, M, L], F32, kind="ExternalOutput")
    xph_d = nc.dram_tensor("xph", [BC, M, 12], F32, kind="ExternalOutput")
    zt_d = nc.dram_tensor("zt", [RSEQ, L], F32, kind="ExternalOutput")

    with tile.TileContext(nc) as tc:
        import contextlib
        ctx = contextlib.ExitStack()
        cp = ctx.enter_context(tc.tile_pool(name="const", bufs=1))
        xp = ctx.enter_context(tc.tile_pool(name="xin", bufs=4))
        hp = ctx.enter_context(tc.tile_pool(name="hact", bufs=2))
        zpool = ctx.enter_context(tc.tile_pool(name="zpool", bufs=4))
        op = ctx.enter_context(tc.tile_pool(name="outs", bufs=3))
        accp = ctx.enter_context(tc.tile_pool(name="acc", bufs=2))
        psp = ctx.enter_context(tc.tile_pool(name="ps", bufs=8, space="PSUM"))

        bal = _EngineBalancer(nc)

        # ---- constants ----
        ident = cp.tile([128, 128], F32)
        make_identity(nc, ident)
        w0e = cp.tile([13, 512], F32R)
        w1e = cp.tile([128, 4, 2, 128], F32R)
        w2ep = cp.tile([128, 2, LP], F32R)
        w2er = cp.tile([128, 2, LR], F32R)
        w0dp = cp.tile([LP, 2, 128], F32R)
        w0dr = cp.tile([LR, 2, 128], F32R)
        w0du = cp.tile([128, 512], F32R)
        w1d = cp.tile([128, 4, 2, 128], F32R)
        w2dp = cp.tile([128, 2, 3], F32R)
        w2dr = cp.tile([128, 2, 9], F32R)
        aT = cp.tile([L, L], F32R)
        aTp = cp.tile([LP, L], F32R)   # A_w.T[0:32]
        aTr = cp.tile([LR, L], F32R)   # A_w.T[32:128]
        bT = cp.tile([CD, L], F32R)
        b_h1e = cp.tile([128, 4], F32)
        b_zp = cp.tile([LP, 1], F32)
        b_zr = cp.tile([LR, 1], F32)
        b_h0d = cp.tile([128, 4], F32)
        b_h1d = cp.tile([128, 4], F32)
        b_xhp = cp.tile([3, 1], F32)
        b_xhr = cp.tile([9, 1], F32)
        for t, d in [(w0e, w0e_d), (w1e, w1e_d), (w2ep, w2ep_d), (w2er, w2er_d),
                     (w0dp, w0dp_d), (w0dr, w0dr_d), (w0du, w0du_d),
                     (w1d, w1d_d), (w2dp, w2dp_d), (w2dr, w2dr_d),
                     (aT, aT_d), (bT, bT_d), (b_h1e, b_h1e_d),
                     (b_zp, b_zp_d), (b_zr, b_zr_d),
                     (b_h0d, b_h0d_d), (b_h1d, b_h1d_d), (b_xhp, b_xhp_d),
                     (b_xhr, b_xhr_d)]:
            nc.sync.dma_start(out=t, in_=d.ap())
        nc.sync.dma_start(out=aTp, in_=aT_d.ap()[0:LP, :])
        nc.sync.dma_start(out=aTr, in_=aT_d.ap()[LP:128, :])

        zk_p = [cp.tile([LP, TS], F32R, name=f"zkp{i}") for i in range(NBB)]
        zk_r = [cp.tile([LR, TS], F32R, name=f"zkr{i}") for i in range(NBB)]

        zt_view = zt_d.ap().rearrange("(t c p) l -> t p c l", c=4, p=128)
        xth_view = xth_d.ap().rearrange("(t c p) f -> t p c f", c=4, p=128)
        xkh_view = xkh_d.ap().rearrange("(t c p) f -> t p c f", c=4, p=128)
        zpd_view = zp_d.ap().rearrange("(bb c p) m l -> bb m p c l", c=4, p=128)
        xph_view = xph_d.ap().rearrange("(bb c p) m f -> bb p c m f", c=4, p=128)
        uT_view = uT_d.ap().rearrange("c m (bb b) -> bb m c b", bb=NBB)

        # ---------- shared MLP machinery ----------
        def dec_h1(h0, tag):
            h1 = hp.tile([128, 4, TS], F32R, name=f"h1d_{tag}", tag="h1d")
            for c in range(4):
                s = c // 2  # 0 = pos side, 1 = rest side
                ps = psp.tile([128, TS], F32, name=f"psh1d{c}_{tag}", tag="pp")
                nc.tensor.matmul(ps, w1d[:, c, 0, :], h0[:, 2 * s + 0, :],
                                 start=True, stop=False)
                nc.tensor.matmul(ps, w1d[:, c, 1, :], h0[:, 2 * s + 1, :],
                                 start=False, stop=True)
                bal.ep(h1[:, c, :], ps, relu=True, bias=b_h1d[:, c:c + 1])
            psx_p = psp.tile([3, TS], F32, name=f"psxhp_{tag}", tag="pp")
            nc.tensor.matmul(psx_p, w2dp[:, 0, :], h1[:, 0, :], start=True, stop=False)
            nc.tensor.matmul(psx_p, w2dp[:, 1, :], h1[:, 1, :], start=False, stop=True)
            psx_r = psp.tile([9, TS], F32, name=f"psxhr_{tag}", tag="pp")
            nc.tensor.matmul(psx_r, w2dr[:, 0, :], h1[:, 2, :], start=True, stop=False)
            nc.tensor.matmul(psx_r, w2dr[:, 1, :], h1[:, 3, :], start=False, stop=True)
            xhp = op.tile([3, TS], F32R, name=f"xhp_{tag}", tag="xhp")
            xhr = op.tile([9, TS], F32R, name=f"xhr_{tag}", tag="xhr")
            bal.ep(xhp, psx_p, bias=b_xhp[:, :])
            bal.ep(xhr, psx_r, bias=b_xhr[:, :])
            return xhp, xhr

        def decoder_split(z_pos, z_rest, tag):
            h0 = hp.tile([128, 4, TS], F32R, name=f"h0d_{tag}", tag="h0d")
            for c in range(4):
                ps = psp.tile([128, TS], F32, name=f"psh0d{c}_{tag}", tag="pp")
                if c < 2:
                    nc.tensor.matmul(ps, w0dp[:, c, :], z_pos,
                                     start=True, stop=True)
                else:
                    nc.tensor.matmul(ps, w0dr[:, c - 2, :], z_rest,
                                     start=True, stop=True)
                bal.ep(h0[:, c, :], ps, relu=True, bias=b_h0d[:, c:c + 1])
            return dec_h1(h0, tag)

        def decoder_unified(z, tag):
            h0 = hp.tile([128, 4, TS], F32R, name=f"h0d_{tag}", tag="h0d")
            for c in range(4):
                ps = psp.tile([128, TS], F32, name=f"psh0d{c}_{tag}", tag="pp")
                nc.tensor.matmul(ps, w0du[:, c * 128:(c + 1) * 128], z,
                                 start=True, stop=True)
                bal.ep(h0[:, c, :], ps, relu=True, bias=b_h0d[:, c:c + 1])
            return dec_h1(h0, tag)

        def xhat_transpose(xhp, xhr, tag, out_ap=None):
            ps = psp.tile([128, 4, 16], F32, name=f"psxt_{tag}", tag="pp")
            for c in range(4):
                nc.tensor.transpose(ps[:, c, 0:3],
                                    xhp.bitcast(F32)[:, c * 128:(c + 1) * 128],
                                    ident[0:3, 0:3])
                nc.tensor.transpose(ps[:, c, 3:12],
                                    xhr.bitcast(F32)[:, c * 128:(c + 1) * 128],
                                    ident[0:9, 0:9])
            if out_ap is None:
                xt = op.tile([128, 4, 12], F32, name=f"xt_{tag}", tag="xt")
                bal.ep(xt, ps[:, :, 0:12])
                return xt
            bal.ep(out_ap, ps[:, :, 0:12])
            return None

        def z_transpose_split(z_pos, z_rest, tag):
            ps = psp.tile([128, 4, 128], F32, name=f"pszt_{tag}", tag="pp")
            for c in range(4):
                nc.tensor.transpose(ps[:, c, 0:LP],
                                    z_pos.bitcast(F32)[:, c * 128:(c + 1) * 128],
                                    ident[0:LP, 0:LP])
                nc.tensor.transpose(ps[:, c, LP:128],
                                    z_rest.bitcast(F32)[:, c * 128:(c + 1) * 128],
                                    ident[0:LR, 0:LR])
            zt = op.tile([128, 4, 128], F32, name=f"zt_{tag}", tag="ztb")
            bal.ep(zt, ps)
            return zt

        def z_transpose_unified(z, tag):
            ps = psp.tile([128, 4, 128], F32, name=f"pszt_{tag}", tag="pp")
            for c in range(4):
                nc.tensor.transpose(ps[:, c, :],
                                    z.bitcast(F32)[:, c * 128:(c + 1) * 128],
                                    ident)
            zt = op.tile([128, 4, 128], F32, name=f"zt_{tag}", tag="ztb")
            bal.ep(zt, ps)
            return zt

        def encoder_tile(src_view, tag, zp_out, zr_out):
            xT = xp.tile([13, TS], F32R, name=f"xT_{tag}", tag="xT")
            nc.sync.dma_start(out=xT, in_=src_view)
            h0 = hp.tile([128, 4, TS], F32R, name=f"h0e_{tag}", tag="h0e")
            for c in range(4):
                ps = psp.tile([128, TS], F32, name=f"psh0e{c}_{tag}", tag="pp")
                nc.tensor.matmul(ps, w0e[:, c * 128:(c + 1) * 128], xT,
                                 start=True, stop=True)
                bal.ep(h0[:, c, :], ps, relu=True)
            h1 = hp.tile([128, 4, TS], F32R, name=f"h1e_{tag}", tag="h1e")
            for c in range(4):
                s = c // 2
                ps = psp.tile([128, TS], F32, name=f"psh1e{c}_{tag}", tag="pp")
                nc.tensor.matmul(ps, w1e[:, c, 0, :], h0[:, 2 * s + 0, :],
                                 start=True, stop=False)
                nc.tensor.matmul(ps, w1e[:, c, 1, :], h0[:, 2 * s + 1, :],
                                 start=False, stop=True)
                bal.ep(h1[:, c, :], ps, relu=True, bias=b_h1e[:, c:c + 1])
            ps_zp = psp.tile([LP, TS], F32, name=f"pszp_{tag}", tag="pp")
            nc.tensor.matmul(ps_zp, w2ep[:, 0, :], h1[:, 0, :], start=True, stop=False)
            nc.tensor.matmul(ps_zp, w2ep[:, 1, :], h1[:, 1, :], start=False, stop=True)
            ps_zr = psp.tile([LR, TS], F32, name=f"pszr_{tag}", tag="pp")
            nc.tensor.matmul(ps_zr, w2er[:, 0, :], h1[:, 2, :], start=True, stop=False)
            nc.tensor.matmul(ps_zr, w2er[:, 1, :], h1[:, 3, :], start=False, stop=True)
            bal.ep(zp_out, ps_zp, bias=b_zp[:, :])
            bal.ep(zr_out, ps_zr, bias=b_zr[:, :])

        # ---------- phase 0: x_k tiles ----------
        for t in range(NT_XK):
            tag = f"xk{t}"
            encoder_tile(xkT_d.ap()[:, t * TS:(t + 1) * TS], tag,
                         zk_p[t], zk_r[t])
            xhp, xhr = decoder_split(zk_p[t], zk_r[t], tag)
            xt = xhat_transpose(xhp, xhr, tag)
            nc.sync.dma_start(out=xkh_view[t], in_=xt)

        # ---------- phase 1: sequence tiles ----------
        for t in range(nt_seq):
            tag = f"s{t}"
            z_pos = zpool.tile([LP, TS], F32R, name=f"ztp_{tag}", tag="ztp")
            z_rest = zpool.tile([LR, TS], F32R, name=f"ztr_{tag}", tag="ztr")
            encoder_tile(xT_d.ap()[:, t * TS:(t + 1) * TS], tag, z_pos, z_rest)
            ztile = z_transpose_split(z_pos, z_rest, tag)
            nc.sync.dma_start(out=zt_view[t], in_=ztile)
            xhp, xhr = decoder_split(z_pos, z_rest, tag)
            xt = xhat_transpose(xhp, xhr, tag)
            nc.sync.dma_start(out=xth_view[t], in_=xt)

        # ---------- phase 2: Koopman scan + decode ----------
        for bb in range(NBB):
            acc = accp.tile([128, 4, M, 12], F32, name=f"xpacc{bb}", tag="xpacc")
            pending = None
            prev = None
            for m in range(n_scan + 1):
                if m < n_scan:
                    tag = f"b{bb}m{m}"
                    uT = xp.tile([CD, TS], F32R, name=f"uT_{tag}", tag="uTt")
                    nc.sync.dma_start(out=uT, in_=uT_view[bb, m])
                    ps_z = psp.tile([128, TS], F32, name=f"psz_{tag}", tag="pp")
                    if m == 0:
                        nc.tensor.matmul(ps_z, aTp, zk_p[bb], start=True, stop=False)
                        nc.tensor.matmul(ps_z, aTr, zk_r[bb], start=False, stop=False)
                    else:
                        nc.tensor.matmul(ps_z, aT, prev, start=True, stop=False)
                    nc.tensor.matmul(ps_z, bT, uT, start=False, stop=True)
                    z_cur = zpool.tile([128, TS], F32R, name=f"zc_{tag}", tag="zu")
                    bal.ep(z_cur, ps_z)
                    prev = z_cur
                if pending is not None:
                    pz, pm, ptag = pending
                    ztile = z_transpose_unified(pz, ptag)
                    nc.sync.dma_start(out=zpd_view[bb, pm], in_=ztile)
                    xhp, xhr = decoder_unified(pz, ptag)
                    xhat_transpose(xhp, xhr, ptag, out_ap=acc[:, :, pm, :])
                if m < n_scan:
                    pending = (z_cur, m, tag)
            nc.sync.dma_start(out=xph_view[bb], in_=acc)

        ctx.close()

    nc.finalize()
    return nc


_CACHED = {}


def _get_module():
    if "nc" not in _CACHED:
        _CACHED["nc"] = build_module()
    return _CACHED["nc"]


def _prep_shared(kw):
    """Host-side weight repacking (shared across cores)."""
    g = {k: np.asarray(v, dtype=np.float32) for k, v in kw.items()
         if not hasattr(v, "keys")}
    w0e = np.zeros((13, 512), np.float32)
    w0e[0:3, 0:256] = g["ep_w0"].T
    w0e[12, 0:256] = g["ep_b0"]
    w0e[3:12, 256:512] = g["er_w0"].T
    w0e[12, 256:512] = g["er_b0"]

    def w1_pack(wp, wr):
        out = np.zeros((128, 4, 2, 128), np.float32)
        for side, w in ((0, wp), (1, wr)):
            wT = w.T  # [256 in, 256 out]
            for n in range(2):
                for k in range(2):
                    out[:, 2 * side + n, k, :] = \
                        wT[k * 128:(k + 1) * 128, n * 128:(n + 1) * 128]
        return out

    def w2_pack(w):  # w: [out, 256] -> [128, 2, out]
        wT = w.T
        return np.stack([wT[0:128], wT[128:256]], axis=1)

    w0du = np.zeros((128, 512), np.float32)
    w0du[0:LP, 0:256] = g["dp_w0"].T
    w0du[LP:128, 256:512] = g["dr_w0"].T
    w0dp = g["dp_w0"].T.reshape(LP, 2, 128)
    w0dr = g["dr_w0"].T.reshape(LR, 2, 128)

    shared = {
        "w0e": w0e,
        "w1e": w1_pack(g["ep_w1"], g["er_w1"]),
        "w2ep": w2_pack(g["ep_w2"]),
        "w2er": w2_pack(g["er_w2"]),
        "w0dp": w0dp,
        "w0dr": w0dr,
        "w0du": w0du,
        "w1d": w1_pack(g["dp_w1"], g["dr_w1"]),
        "w2dp": w2_pack(g["dp_w2"]),
        "w2dr": w2_pack(g["dr_w2"]),
        "aT": g["A_w"].T,
        "bT": g["B_w"].T,
    }
    shared = {k: _round_f32r(v) for k, v in shared.items()}
    shared.update({
        "b_h1e": np.stack([g["ep_b1"][0:128], g["ep_b1"][128:256],
                           g["er_b1"][0:128], g["er_b1"][128:256]], axis=1),
        "b_zp": g["ep_b2"][:, None],
        "b_zr": g["er_b2"][:, None],
        "b_h0d": np.stack([g["dp_b0"][0:128], g["dp_b0"][128:256],
                           g["dr_b0"][0:128], g["dr_b0"][128:256]], axis=1),
        "b_h1d": np.stack([g["dp_b1"][0:128], g["dp_b1"][128:256],
                           g["dr_b1"][0:128], g["dr_b1"][128:256]], axis=1),
        "b_xhp": g["dp_b2"][:, None],
        "b_xhr": g["dr_b2"][:, None],
    })
    return {k: np.ascontiguousarray(v, np.float32) for k, v in shared.items()}


def kernel(**inputs):
    x_k = np.asarray(inputs["x_k"], np.float32)
    u_seq = np.asarray(inputs["u_seq"], np.float32)
    x_next = np.asarray(inputs["x_next_seq"], np.float32)
    shared = _prep_shared(inputs)

    in_maps = []
    for c in range(NCORES):
        sl = slice(c * BC, (c + 1) * BC)
        xn = x_next[sl].reshape(RSEQ, 12)
        xT = np.empty((13, RSEQ), np.float32)
        xT[0:12] = xn.T
        xT[12] = 1.0
        xkT = np.empty((13, BC), np.float32)
        xkT[0:12] = x_k[sl].T
        xkT[12] = 1.0
        uT = np.ascontiguousarray(u_seq[sl].transpose(2, 1, 0))  # [CD, M, BC]
        mm = dict(shared)
        mm["xT"] = _round_f32r(xT)
        mm["xkT"] = _round_f32r(xkT)
        mm["uT"] = _round_f32r(uT)
        in_maps.append(mm)

    nc = _get_module()
    res = run_bass_kernel_spmd(nc, in_maps, core_ids=list(range(NCORES)))
    _CACHED["last_res"] = res

    x_k_hat = np.concatenate([r["xkh"] for r in res.results], 0)
    x_t_hat = np.concatenate(
        [r["xth"].reshape(BC, M, 12) for r in res.results], 0)
    z_pred = np.concatenate([r["zp"] for r in res.results], 0)
    x_p_hat = np.concatenate([r["xph"] for r in res.results], 0)
    z_targ = np.concatenate(
        [r["zt"].reshape(BC, M, L) for r in res.results], 0)
    return (x_k_hat, x_t_hat, z_pred, x_p_hat, z_targ)
